# revision 9
# baseline (speedup 1.0000x reference)
"""Trainium2 Bass kernel for nn_BlockAttnResTransformerBlock.

Two sequential inter-block-attention sub-layers over 9 blocks (8 completed +
partial). Per token t: logits_n = <q, V_n[t]> * rsqrt(mean(V_n[t]^2)+eps)/32,
alpha = softmax_n, h = sum_n alpha_n V_n[t], out = partial + rmsnorm(h) @ W^T.

Engine split (per core: 8 token-tiles of 128, two phases):
  PE   : dots (q-as-weights over d-major V), 5/9 h-build blocks (diag matmuls),
         the DxD GEMM.
  DVE  : softmax glue, Quake rsqrt, 4/9 h-build blocks (scalar_tensor_tensor
         FMA chain, final op folds in the PE PSUM partial), fused output
         (g*rinv + partial), partial-block phase-2 stats (tensor_tensor_reduce).
  ACT  : per-block sum-of-squares (Square + accum), exp, h sum-of-squares.
  DMA  : V in two layouts (natural + d-major, bf16), weights, tiny dot
         transposes, h transposes, output.

rinv(h) is applied at the GEMM *output* (fused into the residual add), so h
enters the GEMM unnormalized. The fp32 partial input is dropped: the residual
base is the bf16 partial (error ~3e-4 of absmax).
"""

import numpy as np
import ml_dtypes
from contextlib import ExitStack

import concourse.bass as bass
import concourse.bacc as bacc
import concourse.tile as tile
from concourse import mybir
from concourse.bass_utils import run_bass_kernel_spmd
from concourse.masks import make_identity

bf16 = ml_dtypes.bfloat16

N_BLK = 8          # completed blocks
NB = 9             # incl. partial
B, T, D = 4, 2048, 1024
NCORES = 8
TOK = B * T                  # 8192
TPC = TOK // NCORES          # 1024 tokens per core
NT = TPC // 128              # 8 token-tiles per core
NCH = D // 128               # 8 d-chunks
EPS = 1e-6
INV_SCALE = 1.0 / 32.0       # 1/sqrt(D)
import os
NPE = int(os.environ.get("K_NPE", "8"))   # h-build blocks on PE; rest on DVE
USE_QW = os.environ.get("K_QW", "1") == "1"  # dots via q-as-weights + transpose
USE_STT_OUT = os.environ.get("K_STT_OUT", "1") == "1"
USE_TTR = os.environ.get("K_TTR", "1") == "1"

_BF = mybir.dt.bfloat16
_F32 = mybir.dt.float32
_I32 = mybir.dt.int32

_CACHE = {}


def build_nc():
    nc = bacc.Bacc("TRN2", target_bir_lowering=False, debug=False)

    vn = nc.dram_tensor("vn", [NT, 128, NB, D], _BF, kind="ExternalInput")
    vt = nc.dram_tensor("vt", [NT, 128, NCH, NB * 128], _BF, kind="ExternalInput")
    qp = nc.dram_tensor("qp", [128, NCH, 16], _BF, kind="ExternalInput")
    qm = nc.dram_tensor("qm", [D], _BF, kind="ExternalInput")
    wa = nc.dram_tensor("wa", [128, NCH, D], _BF, kind="ExternalInput")
    wm = nc.dram_tensor("wm", [128, NCH, D], _BF, kind="ExternalInput")
    out = nc.dram_tensor("out", [NT, 128, D], _F32, kind="ExternalOutput")

    AF = mybir.ActivationFunctionType
    AX = mybir.AxisListType
    OP = mybir.AluOpType

    with tile.TileContext(nc) as tc, ExitStack() as ctx:
        consts = ctx.enter_context(tc.tile_pool(name="consts", bufs=1))
        vin = ctx.enter_context(tc.tile_pool(name="vin", bufs=2))
        stats = ctx.enter_context(tc.tile_pool(name="stats", bufs=2))
        work = ctx.enter_context(tc.tile_pool(name="work", bufs=2))
        pps = ctx.enter_context(tc.tile_pool(name="pps", bufs=2))
        pdot = ctx.enter_context(tc.tile_pool(name="pdot", bufs=2, space="PSUM"))
        ph = ctx.enter_context(tc.tile_pool(name="ph", bufs=2, space="PSUM"))
        pg = ctx.enter_context(tc.tile_pool(name="pg", bufs=1, space="PSUM"))

        ident = consts.tile([128, 128], _BF)
        make_identity(nc, ident)
        qp_sb = consts.tile([128, NCH, 16], _BF)
        nc.sync.dma_start(out=qp_sb, in_=qp[:, :, :])
        qm_bc = consts.tile([128, D], _BF)
        qm_ap = qm[:]
        nc.sync.dma_start(out=qm_bc, in_=bass.AP(
            tensor=qm_ap.tensor, offset=qm_ap.offset, ap=[[0, 128]] + list(qm_ap.ap)))
        wa_sb = consts.tile([128, NCH, D], _BF)
        nc.sync.dma_start(out=wa_sb, in_=wa[:, :, :])
        wm_sb = consts.tile([128, NCH, D], _BF)
        nc.sync.dma_start(out=wm_sb, in_=wm[:, :, :])

        def quake_rsqrt(src_ap, w, pool, tagp):
            """y ~= rsqrt(src/D + eps) on DVE (Quake seed + 2 Newton iters)."""
            ms = pool.tile([128, w], _F32, tag=tagp + "ms")
            nc.vector.tensor_scalar(out=ms, in0=src_ap, scalar1=1.0 / D,
                                    scalar2=EPS, op0=OP.mult, op1=OP.add)
            i32 = pool.tile([128, w], _I32, tag=tagp + "i")
            nc.vector.tensor_scalar(out=i32, in0=ms.bitcast(_I32), scalar1=1,
                                    scalar2=-1, op0=OP.logical_shift_right,
                                    op1=OP.bitwise_xor)
            y0i = pool.tile([128, w], _I32, tag=tagp + "y0")
            nc.vector.tensor_scalar(out=y0i, in0=i32, scalar1=1597463008,
                                    scalar2=None, op0=OP.add)
            ycur = y0i.bitcast(_F32)
            t1 = pool.tile([128, w], _F32, tag=tagp + "t")
            for it in range(2):
                ynext = pool.tile([128, w], _F32, tag=tagp + f"y{it}")
                nc.vector.tensor_mul(out=t1, in0=ycur, in1=ycur)
                nc.vector.tensor_mul(out=t1, in0=t1, in1=ms)
                nc.vector.tensor_scalar(out=t1, in0=t1, scalar1=-0.5,
                                        scalar2=1.5, op0=OP.mult, op1=OP.add)
                nc.vector.tensor_mul(out=ynext, in0=ycur, in1=t1)
                ycur = ynext
            return ycur

        for tt in range(NT):
            v_sb = vin.tile([128, NB, D], _BF)
            nc.sync.dma_start(out=v_sb, in_=vn[tt])
            vt_sb = vin.tile([128, NCH, NB * 128], _BF)
            nc.sync.dma_start(out=vt_sb, in_=vt[tt])

            # --- dots for 9 blocks x 2 queries ------------------------------
            if USE_QW:
                # q-as-weights: out [16(q,pad), n*t] in 3 groups of 3 blocks,
                # bf16-ified on ACT, then xbar-transposed to [128t, n, q]
                dTf = work.tile([128, NB, 16], _BF, tag="dTf")
                for g in range(3):
                    d_ps = pdot.tile([16, 384], _F32, tag="dg")
                    for c in range(NCH):
                        nc.tensor.matmul(
                            d_ps, lhsT=qp_sb[:, c, :],
                            rhs=vt_sb[:, c, g * 384:(g + 1) * 384],
                            start=(c == 0), stop=(c == NCH - 1))
                    dcp = work.tile([16, 3, 128], _BF, tag="dcp")
                    nc.scalar.copy(out=dcp,
                                   in_=d_ps.rearrange("p (n t) -> p n t", n=3))
                    for k in range(3):
                        nc.sync.dma_start_transpose(dTf[:, 3 * g + k, :],
                                                    dcp[:, k, :])
                dT = dTf.rearrange("p n j -> p j n")          # [128, q, n]
            else:
                d_ps = pdot.tile([128, 2 * NB], _F32, tag="dps")
                for n in range(NB):
                    for c in range(NCH):
                        nc.tensor.matmul(d_ps[:, 2 * n:2 * n + 2],
                                         lhsT=vt_sb[:, c, n * 128:(n + 1) * 128],
                                         rhs=qp_sb[:, c, 0:2],
                                         start=(c == 0), stop=(c == NCH - 1))
                dsb = work.tile([128, 2 * NB], _F32, tag="dsb")
                nc.vector.tensor_copy(out=dsb, in_=d_ps)
                dT = dsb.rearrange("p (n j) -> p j n", j=2)   # [128, q, n]

            # --- per-block sum of squares (ACT square + accumulate) ---------
            ssq = stats.tile([128, NB], _F32, tag="ssq")
            junk = work.tile([128, D], _BF, tag="junk")
            for n in range(NB):
                nc.scalar.activation(out=junk, in_=v_sb[:, n, :], func=AF.Square,
                                     accum_out=ssq[:, n:n + 1])
            rinv = quake_rsqrt(ssq[:, :], NB, stats, "r9")

            p1f = pps.tile([128, D], _F32, tag="p1f")
            p1b = pps.tile([128, D], _BF, tag="p1b")

            for phase in range(2):
                w_sb = wa_sb if phase == 0 else wm_sb
                pblk = v_sb[:, 8, :] if phase == 0 else p1b

                # logits = dot * rinv ; softmax over the 9 blocks
                lg = stats.tile([128, NB], _F32, tag="lg")
                if phase == 0:
                    nc.vector.tensor_mul(out=lg, in0=dT[:, 0, :], in1=rinv)
                else:
                    # blocks 0..7 reuse precomputed stats; block 8 is p1
                    nc.vector.tensor_mul(out=lg[:, 0:8], in0=dT[:, 1, 0:8],
                                         in1=rinv[:, 0:8])
                    jj = work.tile([128, D], _BF, tag="junk")
                    pdot2 = stats.tile([128, 1], _F32, tag="pd2")
                    p1sq = stats.tile([128, 1], _F32, tag="p1sq")
                    if USE_TTR:
                        nc.vector.scalar_tensor_tensor(
                            out=jj, in0=p1b, scalar=1.0, in1=qm_bc,
                            op0=OP.mult, op1=OP.mult, accum_out=pdot2)
                        jjs = work.tile([128, D], _BF, tag="junk")
                        nc.scalar.activation(out=jjs, in_=p1b, func=AF.Square,
                                             accum_out=p1sq)
                    else:
                        nc.vector.tensor_mul(out=jj, in0=p1b, in1=qm_bc)
                        nc.vector.tensor_reduce(out=pdot2, in_=jj, axis=AX.X,
                                                op=OP.add)
                        jj2 = work.tile([128, D], _BF, tag="junk")
                        nc.vector.tensor_mul(out=jj2, in0=p1b, in1=p1b)
                        nc.vector.tensor_reduce(out=p1sq, in_=jj2, axis=AX.X,
                                                op=OP.add)
                    ri8 = quake_rsqrt(p1sq[:, :], 1, stats, "r8")
                    nc.vector.tensor_mul(out=lg[:, 8:9], in0=pdot2, in1=ri8)

                mx = stats.tile([128, 1], _F32, tag="mx")
                nc.vector.reduce_max(out=mx, in_=lg, axis=AX.X)
                mb = stats.tile([128, 1], _F32, tag="mb")
                nc.vector.tensor_scalar_mul(out=mb, in0=mx, scalar1=-INV_SCALE)
                ex = stats.tile([128, NB], _F32, tag="ex")
                se = stats.tile([128, 1], _F32, tag="se")
                nc.scalar.activation(out=ex, in_=lg, func=AF.Exp,
                                     scale=INV_SCALE, bias=mb[:, :], accum_out=se)
                rs = stats.tile([128, 1], _F32, tag="rs")
                nc.vector.reciprocal(out=rs, in_=se)
                alpha = stats.tile([128, NB], _F32, tag="alpha")
                nc.vector.tensor_scalar_mul(out=alpha, in0=ex, scalar1=rs)

                # h = sum_n alpha_n * V_n : blocks 0..NPE-1 on PE (diag),
                # NPE..7 + partial on DVE (STT chain, final folds PSUM in)
                npe = min(NPE, 9)
                diag = work.tile([128, npe, 128], _BF, tag="diag")
                for n in range(npe):
                    nc.vector.tensor_scalar_mul(out=diag[:, n, :], in0=ident,
                                                scalar1=alpha[:, n:n + 1])
                h_ps = ph.tile([128, D], _F32, tag="hps")
                for half in range(2):
                    sl = slice(512 * half, 512 * half + 512)
                    for n in range(npe):
                        rhs = v_sb[:, n, sl] if n < 8 else pblk[:, sl]
                        nc.tensor.matmul(h_ps[:, sl], lhsT=diag[:, n, :],
                                         rhs=rhs,
                                         start=(n == 0), stop=(n == npe - 1))
                h = work.tile([128, D], _BF, tag="h")
                if NPE >= 9:
                    nc.vector.tensor_copy(out=h, in_=h_ps)
                else:
                    nc.vector.tensor_scalar_mul(out=h, in0=v_sb[:, NPE, :] if NPE < 8 else pblk,
                                                scalar1=alpha[:, NPE:NPE + 1])
                    for n in range(NPE + 1, 8):
                        nc.vector.scalar_tensor_tensor(
                            out=h, in0=v_sb[:, n, :], scalar=alpha[:, n:n + 1],
                            in1=h, op0=OP.mult, op1=OP.add)
                    if NPE < 8:
                        nc.vector.scalar_tensor_tensor(
                            out=h, in0=pblk, scalar=alpha[:, 8:9], in1=h,
                            op0=OP.mult, op1=OP.add)
                    nc.vector.tensor_add(out=h, in0=h, in1=h_ps)

                # rsqrt(mean(h^2)): ACT square-accum, DVE quake
                hsq = stats.tile([128, 1], _F32, tag="hsq")
                junk2 = work.tile([128, D], _BF, tag="junk")
                nc.scalar.activation(out=junk2, in_=h, func=AF.Square,
                                     accum_out=hsq)
                rih = quake_rsqrt(hsq[:, :], 1, stats, "rh")

                # GEMM on unnormalized h; rinv folded into the output STT
                hT = work.tile([128, NCH, 128], _BF, tag="hT")
                nc.sync.dma_start_transpose(hT, h)
                g_ps = pg.tile([128, D], _F32, tag="gps")
                for half in range(2):
                    sl = slice(512 * half, 512 * half + 512)
                    for c in range(NCH):
                        nc.tensor.matmul(g_ps[:, sl], lhsT=hT[:, c, :],
                                         rhs=w_sb[:, c, sl],
                                         start=(c == 0), stop=(c == NCH - 1))

                dst = p1f if phase == 0 else pps.tile([128, D], _F32, tag="ob")
                res = v_sb[:, 8, :] if phase == 0 else p1f
                if USE_STT_OUT:
                    nc.vector.scalar_tensor_tensor(
                        out=dst, in0=g_ps, scalar=rih[:, :], in1=res,
                        op0=OP.mult, op1=OP.add)
                else:
                    gsc = work.tile([128, D], _F32, tag="gsc")
                    nc.vector.tensor_scalar_mul(out=gsc, in0=g_ps,
                                                scalar1=rih[:, :])
                    nc.vector.tensor_add(out=dst, in0=gsc, in1=res)
                if phase == 0:
                    nc.vector.tensor_copy(out=p1b, in_=p1f)
                else:
                    nc.sync.dma_start(out=out[tt], in_=dst)

    nc.compile()
    return nc


def _get_nc():
    if "nc" not in _CACHE:
        _CACHE["nc"] = build_nc()
    return _CACHE["nc"]


def _prepare_in_maps(completed_blocks, partial_block, attn_norm_w, attn_w,
                     mlp_norm_w, mlp_w, attn_res_query, attn_res_norm_w,
                     mlp_res_query, mlp_res_norm_w):
    V = np.ascontiguousarray(np.asarray(completed_blocks, np.float32)).reshape(N_BLK, TOK, D)
    P = np.ascontiguousarray(np.asarray(partial_block, np.float32)).reshape(TOK, D)
    qwa = np.asarray(attn_res_query, np.float32) * np.asarray(attn_res_norm_w, np.float32)
    qwm = np.asarray(mlp_res_query, np.float32) * np.asarray(mlp_res_norm_w, np.float32)
    WaT = (np.asarray(attn_w, np.float32) * np.asarray(attn_norm_w, np.float32)[None, :]).T
    WmT = (np.asarray(mlp_w, np.float32) * np.asarray(mlp_norm_w, np.float32)[None, :]).T

    qp_host = np.zeros((128, NCH, 16), bf16)
    qp_host[:, :, 0] = qwa.astype(bf16).reshape(NCH, 128).T
    qp_host[:, :, 1] = qwm.astype(bf16).reshape(NCH, 128).T
    qm_host = np.ascontiguousarray(qwm.astype(bf16))
    wa_host = np.ascontiguousarray(WaT.astype(bf16).reshape(NCH, 128, D).transpose(1, 0, 2))
    wm_host = np.ascontiguousarray(WmT.astype(bf16).reshape(NCH, 128, D).transpose(1, 0, 2))

    in_maps = []
    for cid in range(NCORES):
        sl = slice(cid * TPC, (cid + 1) * TPC)
        Vc = np.concatenate([V[:, sl, :], P[None, sl, :]], axis=0).astype(bf16)  # [9, 1024, 1024]
        vn_host = np.ascontiguousarray(
            Vc.reshape(NB, NT, 128, D).transpose(1, 2, 0, 3))          # [tt,t,n,d]
        # vt: [tt, d-part(128), chunk, n*t]  from  Vc [n, tt, t, c, dp]
        vt_host = np.ascontiguousarray(
            Vc.reshape(NB, NT, 128, NCH, 128).transpose(1, 4, 3, 0, 2)
            .reshape(NT, 128, NCH, NB * 128))
        in_maps.append(dict(vn=vn_host, vt=vt_host, qp=qp_host, qm=qm_host,
                            wa=wa_host, wm=wm_host))
    return in_maps


def _run(in_maps, **kw):
    nc = _get_nc()
    return run_bass_kernel_spmd(nc, in_maps, core_ids=list(range(NCORES)), **kw)


def kernel(completed_blocks, partial_block, attn_norm_w, attn_w, mlp_norm_w,
           mlp_w, attn_res_query, attn_res_norm_w, mlp_res_query,
           mlp_res_norm_w, layer_in_block=None, **_unused):
    in_maps = _prepare_in_maps(completed_blocks, partial_block, attn_norm_w,
                               attn_w, mlp_norm_w, mlp_w, attn_res_query,
                               attn_res_norm_w, mlp_res_query, mlp_res_norm_w)
    res = _run(in_maps)
    outs = [np.asarray(r["out"], np.float32).reshape(TPC, D) for r in res.results]
    return np.concatenate(outs, axis=0).reshape(B, T, D)
